# revision 8
# baseline (speedup 1.0000x reference)
"""EfficientAttention (linear attention) Trainium2 Bass kernel.

Computes, per batch b:
    q_n = softmax(q[b], axis=-1)        # over feature dim D=64
    k_n = softmax(k[b], axis=-1)
    ctx = k_n^T @ v[b]                  # [D, D]
    out[b] = q_n @ ctx                  # [N, D]

Sharding: batch dim (32) split across 8 cores, 4 batches per core.

Design notes (per core):
- fp16 I/O: the host casts q/k/v to fp16 and the kernel stores fp16
  outputs (cast back to fp32 on host). Halves HBM traffic vs fp32 —
  this kernel is HBM-bound (32 MB @ ~358 GB/s/core ≈ 89 us floor).
- Stage-skewed software pipeline over 10 "rounds" (8 kv blocks + 2
  drain rounds; kv block j and q block j-2 share round j). In round r:
  DMA triggers for blocks of round r+2, elementwise (exp/reduce/
  normalize) for blocks of round r+1, matmuls + output normalize +
  store for blocks of round r. Every matmul issued in a round has its
  inputs ready a round earlier: the in-order PE queue never
  head-of-line blocks, and the PE stays dense (HAM clock stays at
  2.4 GHz).
- DMA: 1 MB loads/stores, >=8 KB contiguous per partition. k loads +
  qT transposes on sync (SP HWDGE ring), v loads + o stores on scalar
  (ACT HWDGE ring): 2 MB per ring per round.
- K/V pass (natural layout, tokens on partitions): exp(k) on ACT,
  row-sums + reciprocal on DVE, normalize on Pool (halves), PE
  accumulates ctx[64,64] via 64 K=128 matmuls per block.
- ctx epilogue: block-diagonal stacked ctxa [128, 130] fp16
  (rows 0:64 = [ctx | 1 | 0], rows 64:128 = [0 | ctx | 1]); memsets on
  Pool, PSUM->SBUF copy on ACT, row duplication via a tiny DMA.
- Q pass via DMA-transpose: q[b] block viewed as [4096 row-pairs, 128]
  is transposed by the DMA xbar during the load -> qT [128, pairs]
  (partitions = 2x64 stacked features of even/odd rows). ACT exp ->
  eq. Matmul chunk c uses strided stationary eq[:, c::32] (pairs
  c+32j) so output partition j holds 64 consecutive rows and the fp16
  store is 8 KB contiguous per partition. Each K=128 matmul against
  ctxa yields [128, 130] = even vals|sum | odd vals|sum. PSUM carved
  as 2-bank tiles (6 chunks): one DVE reciprocal per 6 chunks, one
  multiply per bank -> fp16 out.
"""

import numpy as np

import concourse.bass as bass
import concourse.mybir as mybir
import concourse.tile as tile
from concourse import bacc
from concourse.bass_utils import run_bass_kernel_spmd

B, N, D = 32, 16384, 64
NCORES = 8
BPC = B // NCORES  # batches per core
LOAD = 8192  # rows per DMA block (1 MB fp16)
LT = LOAD // 128  # row-tile slots per load (64)
NBLK = N // LOAD  # load blocks per batch (2)
QP = LOAD // 2  # row-pairs per q block (4096)
CH = 32  # matmul chunks per q block (128 pairs each)
F32 = mybir.dt.float32
F16 = mybir.dt.float16
EXP = mybir.ActivationFunctionType.Exp
COPY = mybir.ActivationFunctionType.Copy

NKV = BPC * NBLK  # kv/q blocks per core (8)
NR = NKV + 2  # rounds


def build_bass():
    nc = bacc.Bacc("TRN2", target_bir_lowering=False, debug=False)
    q = nc.dram_tensor("q", [BPC, N, D], F16, kind="ExternalInput").ap()
    k = nc.dram_tensor("k", [BPC, N, D], F16, kind="ExternalInput").ap()
    v = nc.dram_tensor("v", [BPC, N, D], F16, kind="ExternalInput").ap()
    o = nc.dram_tensor("o", [BPC, N, D], F16, kind="ExternalOutput").ap()

    def blk(t, b, n0):
        return t[b, n0 : n0 + LOAD, :].rearrange("(p t) d -> p t d", p=128)

    def bi(j):
        return divmod(j, NBLK)

    with tile.TileContext(nc) as tc:
        with (
            tc.tile_pool(name="io", bufs=2) as io,
            tc.tile_pool(name="work", bufs=3) as work,
            tc.tile_pool(name="ctxp", bufs=2) as ctxp,
            tc.tile_pool(name="ps_o", bufs=3, space="PSUM") as ps_o,
            tc.tile_pool(name="ps_c", bufs=2, space="PSUM") as ps_c,
        ):
            H = LT // 2  # half-block tiles (32)
            tiles = {}
            ctx_ps = {}
            ctxa = {}

            def dma_kv(j):
                b, i = bi(j)
                n0 = i * LOAD
                k_sb = io.tile([128, LT, 64], F16, tag="k_sb", bufs=4)
                v_sb = io.tile([128, LT, 64], F16, tag="v_sb", bufs=4)
                nc.sync.dma_start(out=k_sb, in_=blk(k, b, n0))
                nc.scalar.dma_start(out=v_sb, in_=blk(v, b, n0))
                tiles["k", j] = k_sb
                tiles["v", j] = v_sb

            def dma_q(j):
                b, i = bi(j)
                qT = io.tile([128, QP], F16, tag="qT", bufs=4, name="qT")
                src = q[b, i * LOAD : (i + 1) * LOAD, :].rearrange(
                    "(r t) d -> r (t d)", t=2
                )
                nc.sync.dma_start_transpose(qT, src)
                tiles["qT", j] = qT

            def ew_kv(j):
                k_sb = tiles.pop(("k", j))
                ek = work.tile([128, LT, 64], F16, tag="ek", bufs=2)
                ekn = work.tile([128, LT, 64], F16, tag="ekn", bufs=3)
                nc.scalar.activation(ek, k_sb, EXP)
                # row sums via fp16 add-tree (TT runs 2x on all-fp16 SBUF;
                # tensor_reduce is capped at 1x) then a short reduce tail
                t1 = work.tile([128, LT, 32], F16, tag="t1")
                t2 = work.tile([128, LT, 16], F16, tag="t2")
                t3 = work.tile([128, LT, 8], F16, tag="t3")
                ks = work.tile([128, LT, 1], F32, tag="ks")
                ksr = work.tile([128, LT, 1], F32, tag="ksr")
                nc.vector.tensor_add(t1, ek[:, :, 0:32], ek[:, :, 32:64])
                nc.vector.tensor_add(t2, t1[:, :, 0:16], t1[:, :, 16:32])
                nc.vector.tensor_add(t3, t2[:, :, 0:8], t2[:, :, 8:16])
                nc.vector.reduce_sum(out=ks, in_=t3, axis=mybir.AxisListType.X)
                nc.vector.reciprocal(ksr, ks)
                nc.gpsimd.tensor_mul(
                    ekn, ek, ksr[:].to_broadcast((128, LT, 64))
                )
                tiles["ekn", j] = ekn

            def ew_q(j):
                qT = tiles.pop(("qT", j))
                eq = work.tile([128, QP], F16, tag="eq", bufs=3)
                nc.scalar.activation(eq, qT, EXP)
                tiles["eq", j] = eq

            def mm_kv(j):
                b, i = bi(j)
                ekn = tiles.pop(("ekn", j))
                v_sb = tiles.pop(("v", j))
                for t in range(LT):
                    nc.tensor.matmul(
                        ctx_ps[b],
                        ekn[:, t, :],
                        v_sb[:, t, :],
                        start=(i == 0 and t == 0),
                        stop=(i == NBLK - 1 and t == LT - 1),
                    )
                if i == NBLK - 1:
                    ca = ctxp.tile([128, 130], F16, tag="ctxa")
                    nc.gpsimd.memset(ca, 0.0)
                    nc.scalar.activation(ca[0:64, 0:64], ctx_ps[b], COPY)
                    nc.gpsimd.memset(ca[0:64, 64:65], 1.0)
                    nc.scalar.dma_start(
                        out=ca[64:128, 65:130], in_=ca[0:64, 0:65]
                    )
                    ctxa[b] = ca

            def q_out(j):
                b, i = bi(j)
                n0 = i * LOAD
                eq = tiles.pop(("eq", j))
                eqa = eq[:]
                pd_eq = eqa.ap[0]
                ca = ctxa[b]
                out_sb = io.tile([128, CH, 2, 64], F16, tag="out_sb", bufs=2)
                groups = [(0, 6), (6, 6), (12, 6), (18, 6), (24, 6), (30, 2)]
                for c0, nch in groups:
                    o_ps = ps_o.tile([128, 2, 512], F32, tag="o_ps")
                    opb = o_ps[:]
                    pdim = opb.ap[0]
                    for s in range(nch):
                        w, sl = divmod(s, 3)
                        lhsT = bass.AP(
                            tensor=eqa.tensor,
                            offset=eqa.offset + c0 + s,
                            ap=[pd_eq, [CH, 128]],
                        )
                        nc.tensor.matmul(
                            o_ps[:, w, sl * 132 : sl * 132 + 130],
                            lhsT,
                            ca,
                            start=True,
                            stop=True,
                        )
                    nbank = (nch + 2) // 3
                    nsl = min(nch, 3)
                    r_sb = work.tile([128, 2, 3, 2, 1], F32, tag="r_sb")
                    rs_ap = bass.AP(
                        tensor=opb.tensor,
                        offset=opb.offset + 64,
                        ap=[pdim, [512, nbank], [132, nsl], [65, 2], [1, 1]],
                    )
                    nc.vector.reciprocal(r_sb[:, 0:nbank, 0:nsl], rs_ap)
                    for w in range(nbank):
                        nw = min(3, nch - 3 * w)
                        vals_ap = bass.AP(
                            tensor=opb.tensor,
                            offset=opb.offset + w * 512,
                            ap=[pdim, [132, nw], [65, 2], [1, 64]],
                        )
                        nc.vector.tensor_mul(
                            out_sb[:, c0 + 3 * w : c0 + 3 * w + nw, :, :],
                            vals_ap,
                            r_sb[:, w, 0:nw].to_broadcast((128, nw, 2, 64)),
                        )
                # partition j holds rows n0+64j .. n0+64j+63 (8 KB contig)
                dst = o[b, n0 : n0 + LOAD, :].rearrange(
                    "(j c w) d -> j c w d", j=128, w=2
                )
                nc.scalar.dma_start(out=dst, in_=out_sb)

            # ---- stage-skewed pipeline ----
            for b in range(BPC):
                ctx_ps[b] = ps_c.tile([64, 64], F32, tag="ctx_ps", name="ctx_ps")
            dma_kv(0)
            dma_kv(1)
            ew_kv(0)
            for r in range(NR):
                if r + 2 < NKV:
                    dma_kv(r + 2)
                if r < NKV:
                    dma_q(r)  # q block r is consumed in round r+2
                if r + 1 < NKV:
                    ew_kv(r + 1)
                if 0 <= r - 1 < NKV:
                    ew_q(r - 1)
                if 0 <= r - 2 < NKV:
                    q_out(r - 2)
                if r < NKV:
                    mm_kv(r)

    nc.compile()
    return nc


_NC_CACHE = None


def kernel(q: np.ndarray, k: np.ndarray, v: np.ndarray) -> np.ndarray:
    global _NC_CACHE
    if _NC_CACHE is None:
        _NC_CACHE = build_bass()
    nc = _NC_CACHE
    q = np.ascontiguousarray(np.asarray(q), dtype=np.float16)
    k = np.ascontiguousarray(np.asarray(k), dtype=np.float16)
    v = np.ascontiguousarray(np.asarray(v), dtype=np.float16)
    in_maps = [
        {
            "q": q[i * BPC : (i + 1) * BPC],
            "k": k[i * BPC : (i + 1) * BPC],
            "v": v[i * BPC : (i + 1) * BPC],
        }
        for i in range(NCORES)
    ]
    res = run_bass_kernel_spmd(nc, in_maps, core_ids=list(range(NCORES)))
    return np.concatenate(
        [res.results[i]["o"] for i in range(NCORES)], axis=0
    ).astype(np.float32)


# revision 9
# speedup vs baseline: 1.1448x; 1.1448x over previous
"""EfficientAttention (linear attention) Trainium2 Bass kernel.

Computes, per batch b:
    q_n = softmax(q[b], axis=-1)        # over feature dim D=64
    k_n = softmax(k[b], axis=-1)
    ctx = k_n^T @ v[b]                  # [D, D]
    out[b] = q_n @ ctx                  # [N, D]

Sharding: batch dim (32) split across 8 cores, 4 batches per core.

Design notes (per core):
- fp16 I/O: the host casts q/k/v to fp16 and the kernel stores fp16
  outputs (cast back to fp32 on host). Halves HBM traffic vs fp32 —
  this kernel is HBM-bound (32 MB @ ~358 GB/s/core ≈ 89 us floor).
- Stage-skewed software pipeline over 10 "rounds" (8 kv blocks + 2
  drain rounds; kv block j and q block j-2 share round j). In round r:
  DMA triggers for blocks of round r+2, elementwise (exp/reduce/
  normalize) for blocks of round r+1, matmuls + output normalize +
  store for blocks of round r. Every matmul issued in a round has its
  inputs ready a round earlier: the in-order PE queue never
  head-of-line blocks, and the PE stays dense (HAM clock stays at
  2.4 GHz).
- DMA: 1 MB loads/stores, >=8 KB contiguous per partition. k loads +
  qT transposes on sync (SP HWDGE ring), v loads + o stores on scalar
  (ACT HWDGE ring): 2 MB per ring per round.
- K/V pass (natural layout, tokens on partitions): exp(k) on ACT,
  row-sums + reciprocal on DVE, normalize on Pool (halves), PE
  accumulates ctx[64,64] via 64 K=128 matmuls per block.
- ctx epilogue: block-diagonal stacked ctxa [128, 130] fp16
  (rows 0:64 = [ctx | 1 | 0], rows 64:128 = [0 | ctx | 1]); memsets on
  Pool, PSUM->SBUF copy on ACT, row duplication via a tiny DMA.
- Q pass via DMA-transpose: q[b] block viewed as [4096 row-pairs, 128]
  is transposed by the DMA xbar during the load -> qT [128, pairs]
  (partitions = 2x64 stacked features of even/odd rows). ACT exp ->
  eq. Matmul chunk c uses strided stationary eq[:, c::32] (pairs
  c+32j) so output partition j holds 64 consecutive rows and the fp16
  store is 8 KB contiguous per partition. Each K=128 matmul against
  ctxa yields [128, 130] = even vals|sum | odd vals|sum. PSUM carved
  as 2-bank tiles (6 chunks): one DVE reciprocal per 6 chunks, one
  multiply per bank -> fp16 out.
"""

import numpy as np

import concourse.bass as bass
import concourse.mybir as mybir
import concourse.tile as tile
from concourse import bacc
from concourse.bass_utils import run_bass_kernel_spmd

B, N, D = 32, 16384, 64
NCORES = 8
BPC = B // NCORES  # batches per core
LOAD = 8192  # rows per DMA block (1 MB fp16)
LT = LOAD // 128  # row-tile slots per load (64)
NBLK = N // LOAD  # load blocks per batch (2)
QP = LOAD // 2  # row-pairs per q block (4096)
CH = 32  # matmul chunks per q block (128 pairs each)
F32 = mybir.dt.float32
F16 = mybir.dt.float16
EXP = mybir.ActivationFunctionType.Exp
COPY = mybir.ActivationFunctionType.Copy

NKV = BPC * NBLK  # kv/q blocks per core (8)
NR = NKV + 2  # rounds


def build_bass():
    nc = bacc.Bacc("TRN2", target_bir_lowering=False, debug=False)
    q = nc.dram_tensor("q", [BPC, N, D], F16, kind="ExternalInput").ap()
    k = nc.dram_tensor("k", [BPC, N, D], F16, kind="ExternalInput").ap()
    v = nc.dram_tensor("v", [BPC, N, D], F16, kind="ExternalInput").ap()
    o = nc.dram_tensor("o", [BPC, N, D], F16, kind="ExternalOutput").ap()

    def blk(t, b, n0):
        return t[b, n0 : n0 + LOAD, :].rearrange("(p t) d -> p t d", p=128)

    def bi(j):
        return divmod(j, NBLK)

    with tile.TileContext(nc) as tc:
        with (
            tc.tile_pool(name="io", bufs=2) as io,
            tc.tile_pool(name="work", bufs=3) as work,
            tc.tile_pool(name="ctxp", bufs=2) as ctxp,
            tc.tile_pool(name="ps_o", bufs=3, space="PSUM") as ps_o,
            tc.tile_pool(name="ps_c", bufs=2, space="PSUM") as ps_c,
        ):
            H = LT // 2  # half-block tiles (32)
            tiles = {}
            ctx_ps = {}
            ctxa = {}

            def dma_kv(j):
                b, i = bi(j)
                n0 = i * LOAD
                k_sb = io.tile([128, LT, 64], F16, tag="k_sb", bufs=4)
                v_sb = io.tile([128, LT, 64], F16, tag="v_sb", bufs=4)
                nc.sync.dma_start(out=k_sb, in_=blk(k, b, n0))
                nc.sync.dma_start(out=v_sb, in_=blk(v, b, n0))
                tiles["k", j] = k_sb
                tiles["v", j] = v_sb

            def dma_q(j):
                b, i = bi(j)
                qT = io.tile([128, QP], F16, tag="qT", bufs=4, name="qT")
                src = q[b, i * LOAD : (i + 1) * LOAD, :].rearrange(
                    "(r t) d -> r (t d)", t=2
                )
                nc.scalar.dma_start_transpose(qT, src)
                tiles["qT", j] = qT

            def ew_kv(j):
                k_sb = tiles.pop(("k", j))
                ek = work.tile([128, LT, 64], F16, tag="ek", bufs=2)
                ekn = work.tile([128, LT, 64], F16, tag="ekn", bufs=3)
                nc.scalar.activation(ek, k_sb, EXP)
                # row sums via fp16 add-tree (TT runs 2x on all-fp16 SBUF;
                # tensor_reduce is capped at 1x) then a short reduce tail
                t1 = work.tile([128, LT, 32], F16, tag="t1")
                t2 = work.tile([128, LT, 16], F16, tag="t2")
                t3 = work.tile([128, LT, 8], F16, tag="t3")
                ks = work.tile([128, LT, 1], F32, tag="ks")
                ksr = work.tile([128, LT, 1], F32, tag="ksr")
                nc.vector.tensor_add(t1, ek[:, :, 0:32], ek[:, :, 32:64])
                nc.vector.tensor_add(t2, t1[:, :, 0:16], t1[:, :, 16:32])
                nc.vector.tensor_add(t3, t2[:, :, 0:8], t2[:, :, 8:16])
                nc.vector.reduce_sum(out=ks, in_=t3, axis=mybir.AxisListType.X)
                nc.vector.reciprocal(ksr, ks)
                nc.gpsimd.tensor_mul(
                    ekn, ek, ksr[:].to_broadcast((128, LT, 64))
                )
                tiles["ekn", j] = ekn

            def ew_q(j):
                qT = tiles.pop(("qT", j))
                eq = work.tile([128, QP], F16, tag="eq", bufs=3)
                nc.scalar.activation(eq, qT, EXP)
                tiles["eq", j] = eq

            def mm_kv(j):
                b, i = bi(j)
                ekn = tiles.pop(("ekn", j))
                v_sb = tiles.pop(("v", j))
                for t in range(LT):
                    nc.tensor.matmul(
                        ctx_ps[b],
                        ekn[:, t, :],
                        v_sb[:, t, :],
                        start=(i == 0 and t == 0),
                        stop=(i == NBLK - 1 and t == LT - 1),
                    )
                if i == NBLK - 1:
                    ca = ctxp.tile([128, 130], F16, tag="ctxa")
                    nc.gpsimd.memset(ca, 0.0)
                    nc.scalar.activation(ca[0:64, 0:64], ctx_ps[b], COPY)
                    nc.gpsimd.memset(ca[0:64, 64:65], 1.0)
                    nc.scalar.dma_start(
                        out=ca[64:128, 65:130], in_=ca[0:64, 0:65]
                    )
                    ctxa[b] = ca

            def q_out(j):
                b, i = bi(j)
                n0 = i * LOAD
                eq = tiles.pop(("eq", j))
                eqa = eq[:]
                pd_eq = eqa.ap[0]
                ca = ctxa[b]
                out_sb = io.tile([128, CH, 2, 64], F16, tag="out_sb", bufs=2)
                groups = [(0, 6), (6, 6), (12, 6), (18, 6), (24, 6), (30, 2)]
                for c0, nch in groups:
                    o_ps = ps_o.tile([128, 2, 512], F32, tag="o_ps")
                    opb = o_ps[:]
                    pdim = opb.ap[0]
                    for s in range(nch):
                        w, sl = divmod(s, 3)
                        lhsT = bass.AP(
                            tensor=eqa.tensor,
                            offset=eqa.offset + c0 + s,
                            ap=[pd_eq, [CH, 128]],
                        )
                        nc.tensor.matmul(
                            o_ps[:, w, sl * 132 : sl * 132 + 130],
                            lhsT,
                            ca,
                            start=True,
                            stop=True,
                        )
                    nbank = (nch + 2) // 3
                    nsl = min(nch, 3)
                    r_sb = work.tile([128, 2, 3, 2, 1], F32, tag="r_sb")
                    rs_ap = bass.AP(
                        tensor=opb.tensor,
                        offset=opb.offset + 64,
                        ap=[pdim, [512, nbank], [132, nsl], [65, 2], [1, 1]],
                    )
                    nc.vector.reciprocal(r_sb[:, 0:nbank, 0:nsl], rs_ap)
                    for w in range(nbank):
                        nw = min(3, nch - 3 * w)
                        vals_ap = bass.AP(
                            tensor=opb.tensor,
                            offset=opb.offset + w * 512,
                            ap=[pdim, [132, nw], [65, 2], [1, 64]],
                        )
                        nc.vector.tensor_mul(
                            out_sb[:, c0 + 3 * w : c0 + 3 * w + nw, :, :],
                            vals_ap,
                            r_sb[:, w, 0:nw].to_broadcast((128, nw, 2, 64)),
                        )
                # partition j holds rows n0+64j .. n0+64j+63 (8 KB contig)
                dst = o[b, n0 : n0 + LOAD, :].rearrange(
                    "(j c w) d -> j c w d", j=128, w=2
                )
                nc.sync.dma_start(out=dst, in_=out_sb)

            # ---- stage-skewed pipeline ----
            for b in range(BPC):
                ctx_ps[b] = ps_c.tile([64, 64], F32, tag="ctx_ps", name="ctx_ps")
            dma_kv(0)
            dma_kv(1)
            ew_kv(0)
            for r in range(NR):
                if r + 2 < NKV:
                    dma_kv(r + 2)
                if r < NKV:
                    dma_q(r)  # q block r is consumed in round r+2
                if r + 1 < NKV:
                    ew_kv(r + 1)
                if 0 <= r - 1 < NKV:
                    ew_q(r - 1)
                if 0 <= r - 2 < NKV:
                    q_out(r - 2)
                if r < NKV:
                    mm_kv(r)

    nc.compile()
    return nc


_NC_CACHE = None


def kernel(q: np.ndarray, k: np.ndarray, v: np.ndarray) -> np.ndarray:
    global _NC_CACHE
    if _NC_CACHE is None:
        _NC_CACHE = build_bass()
    nc = _NC_CACHE
    q = np.ascontiguousarray(np.asarray(q), dtype=np.float16)
    k = np.ascontiguousarray(np.asarray(k), dtype=np.float16)
    v = np.ascontiguousarray(np.asarray(v), dtype=np.float16)
    in_maps = [
        {
            "q": q[i * BPC : (i + 1) * BPC],
            "k": k[i * BPC : (i + 1) * BPC],
            "v": v[i * BPC : (i + 1) * BPC],
        }
        for i in range(NCORES)
    ]
    res = run_bass_kernel_spmd(nc, in_maps, core_ids=list(range(NCORES)))
    return np.concatenate(
        [res.results[i]["o"] for i in range(NCORES)], axis=0
    ).astype(np.float32)


# revision 10
# speedup vs baseline: 1.1505x; 1.0049x over previous
"""EfficientAttention (linear attention) Trainium2 Bass kernel.

Computes, per batch b:
    q_n = softmax(q[b], axis=-1)        # over feature dim D=64
    k_n = softmax(k[b], axis=-1)
    ctx = k_n^T @ v[b]                  # [D, D]
    out[b] = q_n @ ctx                  # [N, D]

Sharding: batch dim (32) split across 8 cores, 4 batches per core.

Design notes (per core):
- fp16 I/O: the host casts q/k/v to fp16 and the kernel stores fp16
  outputs (cast back to fp32 on host). Halves HBM traffic vs fp32 —
  this kernel is HBM-bound (32 MB @ ~358 GB/s/core ≈ 89 us floor).
- Stage-skewed software pipeline over 10 "rounds" (8 kv blocks + 2
  drain rounds; kv block j and q block j-2 share round j). In round r:
  DMA triggers for blocks of round r+2, elementwise (exp/reduce/
  normalize) for blocks of round r+1, matmuls + output normalize +
  store for blocks of round r. Every matmul issued in a round has its
  inputs ready a round earlier: the in-order PE queue never
  head-of-line blocks, and the PE stays dense (HAM clock stays at
  2.4 GHz).
- DMA: 1 MB loads/stores, >=8 KB contiguous per partition. k loads +
  qT transposes on sync (SP HWDGE ring), v loads + o stores on scalar
  (ACT HWDGE ring): 2 MB per ring per round.
- K/V pass (natural layout, tokens on partitions): exp(k) on ACT,
  row-sums + reciprocal on DVE, normalize on Pool (halves), PE
  accumulates ctx[64,64] via 64 K=128 matmuls per block.
- ctx epilogue: block-diagonal stacked ctxa [128, 130] fp16
  (rows 0:64 = [ctx | 1 | 0], rows 64:128 = [0 | ctx | 1]); memsets on
  Pool, PSUM->SBUF copy on ACT, row duplication via a tiny DMA.
- Q pass via DMA-transpose: q[b] block viewed as [4096 row-pairs, 128]
  is transposed by the DMA xbar during the load -> qT [128, pairs]
  (partitions = 2x64 stacked features of even/odd rows). ACT exp ->
  eq. Matmul chunk c uses strided stationary eq[:, c::32] (pairs
  c+32j) so output partition j holds 64 consecutive rows and the fp16
  store is 8 KB contiguous per partition. Each K=128 matmul against
  ctxa yields [128, 130] = even vals|sum | odd vals|sum. PSUM carved
  as 2-bank tiles (6 chunks): one DVE reciprocal per 6 chunks, one
  multiply per bank -> fp16 out.
"""

import numpy as np

import concourse.bass as bass
import concourse.mybir as mybir
import concourse.tile as tile
from concourse import bacc
from concourse.bass_utils import run_bass_kernel_spmd

B, N, D = 32, 16384, 64
NCORES = 8
BPC = B // NCORES  # batches per core
LOAD = 8192  # rows per DMA block (1 MB fp16)
LT = LOAD // 128  # row-tile slots per load (64)
NBLK = N // LOAD  # load blocks per batch (2)
QP = LOAD // 2  # row-pairs per q block (4096)
CH = 32  # matmul chunks per q block (128 pairs each)
F32 = mybir.dt.float32
F16 = mybir.dt.float16
EXP = mybir.ActivationFunctionType.Exp
COPY = mybir.ActivationFunctionType.Copy

NKV = BPC * NBLK  # kv/q blocks per core (8)
NR = NKV + 2  # rounds


def build_bass():
    nc = bacc.Bacc("TRN2", target_bir_lowering=False, debug=False)
    q = nc.dram_tensor("q", [BPC, N, D], F16, kind="ExternalInput").ap()
    k = nc.dram_tensor("k", [BPC, N, D], F16, kind="ExternalInput").ap()
    v = nc.dram_tensor("v", [BPC, N, D], F16, kind="ExternalInput").ap()
    o = nc.dram_tensor("o", [BPC, N, D], F16, kind="ExternalOutput").ap()

    def blk(t, b, n0):
        return t[b, n0 : n0 + LOAD, :].rearrange("(p t) d -> p t d", p=128)

    def bi(j):
        return divmod(j, NBLK)

    with tile.TileContext(nc) as tc:
        with (
            tc.tile_pool(name="io", bufs=2) as io,
            tc.tile_pool(name="work", bufs=3) as work,
            tc.tile_pool(name="ctxp", bufs=2) as ctxp,
            tc.tile_pool(name="ps_o", bufs=3, space="PSUM") as ps_o,
            tc.tile_pool(name="ps_c", bufs=2, space="PSUM") as ps_c,
        ):
            H = LT // 2  # half-block tiles (32)
            tiles = {}
            ctx_ps = {}
            ctxa = {}

            def dma_kv(j):
                b, i = bi(j)
                n0 = i * LOAD
                k_sb = io.tile([128, LT, 64], F16, tag="k_sb", bufs=4)
                v_sb = io.tile([128, LT, 64], F16, tag="v_sb", bufs=4)
                nc.sync.dma_start(out=k_sb, in_=blk(k, b, n0))
                nc.sync.dma_start(out=v_sb, in_=blk(v, b, n0))
                tiles["k", j] = k_sb
                tiles["v", j] = v_sb

            def dma_q(j):
                b, i = bi(j)
                qT = io.tile([128, QP], F16, tag="qT", bufs=4, name="qT")
                src = q[b, i * LOAD : (i + 1) * LOAD, :].rearrange(
                    "(r t) d -> r (t d)", t=2
                )
                nc.sync.dma_start_transpose(qT, src)
                tiles["qT", j] = qT

            def ew_kv(j):
                k_sb = tiles.pop(("k", j))
                ek = work.tile([128, LT, 64], F16, tag="ek", bufs=2)
                ekn = work.tile([128, LT, 64], F16, tag="ekn", bufs=3)
                nc.scalar.activation(ek, k_sb, EXP)
                # row sums via fp16 add-tree (TT runs 2x on all-fp16 SBUF;
                # tensor_reduce is capped at 1x) then a short reduce tail
                t1 = work.tile([128, LT, 32], F16, tag="t1")
                t2 = work.tile([128, LT, 16], F16, tag="t2")
                t3 = work.tile([128, LT, 8], F16, tag="t3")
                ks = work.tile([128, LT, 1], F32, tag="ks")
                ksr = work.tile([128, LT, 1], F32, tag="ksr")
                nc.vector.tensor_add(t1, ek[:, :, 0:32], ek[:, :, 32:64])
                nc.vector.tensor_add(t2, t1[:, :, 0:16], t1[:, :, 16:32])
                nc.vector.tensor_add(t3, t2[:, :, 0:8], t2[:, :, 8:16])
                nc.vector.reduce_sum(out=ks, in_=t3, axis=mybir.AxisListType.X)
                nc.vector.reciprocal(ksr, ks)
                nc.gpsimd.tensor_mul(
                    ekn, ek, ksr[:].to_broadcast((128, LT, 64))
                )
                tiles["ekn", j] = ekn

            def ew_q(j):
                qT = tiles.pop(("qT", j))
                eq = work.tile([128, QP], F16, tag="eq", bufs=3)
                nc.scalar.activation(eq, qT, EXP)
                tiles["eq", j] = eq

            def mm_kv(j):
                b, i = bi(j)
                ekn = tiles.pop(("ekn", j))
                v_sb = tiles.pop(("v", j))
                for t in range(LT):
                    nc.tensor.matmul(
                        ctx_ps[b],
                        ekn[:, t, :],
                        v_sb[:, t, :],
                        start=(i == 0 and t == 0),
                        stop=(i == NBLK - 1 and t == LT - 1),
                    )
                if i == NBLK - 1:
                    ca = ctxp.tile([128, 130], F16, tag="ctxa")
                    nc.gpsimd.memset(ca, 0.0)
                    nc.scalar.activation(ca[0:64, 0:64], ctx_ps[b], COPY)
                    nc.gpsimd.memset(ca[0:64, 64:65], 1.0)
                    nc.scalar.dma_start(
                        out=ca[64:128, 65:130], in_=ca[0:64, 0:65]
                    )
                    ctxa[b] = ca

            def q_out(j):
                b, i = bi(j)
                n0 = i * LOAD
                eq = tiles.pop(("eq", j))
                eqa = eq[:]
                pd_eq = eqa.ap[0]
                ca = ctxa[b]
                out_sb = io.tile([128, CH, 2, 64], F16, tag="out_sb", bufs=2)
                groups = [(0, 6), (6, 6), (12, 6), (18, 6), (24, 6), (30, 2)]
                for c0, nch in groups:
                    o_ps = ps_o.tile([128, 2, 512], F32, tag="o_ps")
                    opb = o_ps[:]
                    pdim = opb.ap[0]
                    for s in range(nch):
                        w, sl = divmod(s, 3)
                        lhsT = bass.AP(
                            tensor=eqa.tensor,
                            offset=eqa.offset + c0 + s,
                            ap=[pd_eq, [CH, 128]],
                        )
                        nc.tensor.matmul(
                            o_ps[:, w, sl * 132 : sl * 132 + 130],
                            lhsT,
                            ca,
                            start=True,
                            stop=True,
                        )
                    nbank = (nch + 2) // 3
                    nsl = min(nch, 3)
                    r_sb = work.tile([128, 2, 3, 2, 1], F32, tag="r_sb")
                    rs_ap = bass.AP(
                        tensor=opb.tensor,
                        offset=opb.offset + 64,
                        ap=[pdim, [512, nbank], [132, nsl], [65, 2], [1, 1]],
                    )
                    nc.vector.reciprocal(r_sb[:, 0:nbank, 0:nsl], rs_ap)
                    for w in range(nbank):
                        nw = min(3, nch - 3 * w)
                        vals_ap = bass.AP(
                            tensor=opb.tensor,
                            offset=opb.offset + w * 512,
                            ap=[pdim, [132, nw], [65, 2], [1, 64]],
                        )
                        nc.vector.tensor_mul(
                            out_sb[:, c0 + 3 * w : c0 + 3 * w + nw, :, :],
                            vals_ap,
                            r_sb[:, w, 0:nw].to_broadcast((128, nw, 2, 64)),
                        )
                # partition j holds rows n0+64j .. n0+64j+63 (8 KB contig)
                dst = o[b, n0 : n0 + LOAD, :].rearrange(
                    "(j c w) d -> j c w d", j=128, w=2
                )
                nc.scalar.dma_start(out=dst, in_=out_sb)

            # ---- stage-skewed pipeline ----
            for b in range(BPC):
                ctx_ps[b] = ps_c.tile([64, 64], F32, tag="ctx_ps", name="ctx_ps")
            dma_kv(0)
            dma_kv(1)
            ew_kv(0)
            for r in range(NR):
                if r + 2 < NKV:
                    dma_kv(r + 2)
                if r < NKV:
                    dma_q(r)  # q block r is consumed in round r+2
                if r + 1 < NKV:
                    ew_kv(r + 1)
                if 0 <= r - 1 < NKV:
                    ew_q(r - 1)
                if 0 <= r - 2 < NKV:
                    q_out(r - 2)
                if r < NKV:
                    mm_kv(r)

    nc.compile()
    return nc


_NC_CACHE = None


def kernel(q: np.ndarray, k: np.ndarray, v: np.ndarray) -> np.ndarray:
    global _NC_CACHE
    if _NC_CACHE is None:
        _NC_CACHE = build_bass()
    nc = _NC_CACHE
    q = np.ascontiguousarray(np.asarray(q), dtype=np.float16)
    k = np.ascontiguousarray(np.asarray(k), dtype=np.float16)
    v = np.ascontiguousarray(np.asarray(v), dtype=np.float16)
    in_maps = [
        {
            "q": q[i * BPC : (i + 1) * BPC],
            "k": k[i * BPC : (i + 1) * BPC],
            "v": v[i * BPC : (i + 1) * BPC],
        }
        for i in range(NCORES)
    ]
    res = run_bass_kernel_spmd(nc, in_maps, core_ids=list(range(NCORES)))
    return np.concatenate(
        [res.results[i]["o"] for i in range(NCORES)], axis=0
    ).astype(np.float32)
